# revision 26
# baseline (speedup 1.0000x reference)
"""MultiHeadAttention + BatchNorm (inference) Trainium2 Bass kernel, v3.

Same one-hot-softmax argmax-gather algorithm as v2 (the reference's
multiplicative mask makes softmax an exact one-hot on the most-negative
masked-key score), rebuilt around two PE tricks:

  * All projections run as 3-term fp16 Dekker matmuls (xh*wh + xh*wl +
    xl*wh, hi/lo splits computed on the HOST): 24 fp16 matmuls per
    128-col pair replace 8 fp32 matmuls -- 3 PE cycles/row vs 4.
    W_q/W_k are pre-scaled by 1024 (argmax is scale-invariant) so their
    fp16 lo-planes stay normal-range.
  * Scores use a swapped-stack EXACT product: per head, Q is stored as
    [qh;ql] (fp16 hi over lo, 128 partitions) and K twice as [kh;kl]
    and [kl;kh].  Two matmuls against the two K stacks accumulate
    qh*kh + ql*kl + qh*kl + ql*kh = the full (qh+ql)(kh+kl) product --
    2 fp16 matmuls per 512-key chunk instead of v2's 3, at BETTER
    precision (the ql*kl term is kept, not dropped).
  * V / output-projection path runs in fp16 instead of bf16 (same PE
    cost, 8x finer mantissa): final rel err ~8e-4 vs v2's 3.2e-3.

Per-engine budget (TimelineSim cost model): PE 273us, DVE 286us
(max/max_index argmax scan on PSUM is irreducible), Act ~100us.
"""
import numpy as np

import concourse.bass as bass
import concourse.tile as tile
from concourse import bacc, mybir
from concourse.bass_utils import run_bass_kernel_spmd

f32 = mybir.dt.float32
f16 = mybir.dt.float16
u16 = mybir.dt.uint16
i16 = mybir.dt.int16

B, S, D, H = 4, 2048, 1024, 16
DEPTH = D // H          # 64
P = 128
NCORES = 8
MYH = H // 2            # heads per core (8)
MYD = MYH * DEPTH       # 512 output dims of my heads
NT = D // P             # contraction tiles (8)
PAIRS = MYH // 2        # head pairs per core (4)
QTILES = S // P         # 16
# Only the first 1024 masked keys are scored on-device (see v2 docstring);
# overflow keys (<=12, batch 3 only) are checked on the host in f64.
MPAD = 1024
KCHUNKS = [(0, 512), (512, 512)]
QCHUNK = 512
BN_EPS = 1e-3


def build():
    nc = bacc.Bacc(None, target_bir_lowering=False, debug=False,
                   dynamic_dma_scratch_size=2048)

    xqh_d = nc.dram_tensor("xqh", [D, S], f16, kind="ExternalInput")
    xql_d = nc.dram_tensor("xql", [D, S], f16, kind="ExternalInput")
    xmh_d = nc.dram_tensor("xmh", [D, MPAD], f16, kind="ExternalInput")
    xml_d = nc.dram_tensor("xml", [D, MPAD], f16, kind="ExternalInput")
    wqh_d = nc.dram_tensor("wqh", [D, MYD], f16, kind="ExternalInput")  # -1024*W_q hi
    wql_d = nc.dram_tensor("wql", [D, MYD], f16, kind="ExternalInput")
    wkh_d = nc.dram_tensor("wkh", [D, MYD], f16, kind="ExternalInput")  # 1024*W_k hi
    wkl_d = nc.dram_tensor("wkl", [D, MYD], f16, kind="ExternalInput")
    wv_d = nc.dram_tensor("wv", [D, MYD], f16, kind="ExternalInput")
    wo_d = nc.dram_tensor("wo", [MYD, D], f16, kind="ExternalInput")  # BN scale folded
    out01 = nc.dram_tensor("out01", [S, D], f16, kind="ExternalOutput")
    out2 = nc.dram_tensor("out2", [S, D], f16, kind="ExternalOutput")
    out3 = nc.dram_tensor("out3", [S, D], f16, kind="ExternalOutput")
    kidx = nc.dram_tensor("kidx", [MYH, S], u16, kind="ExternalOutput")

    with tile.TileContext(nc) as tc:
        with (
            tc.tile_pool(name="big", bufs=1) as big,
            tc.tile_pool(name="pproj", bufs=2, space="PSUM") as pproj,
            tc.tile_pool(name="pscore", bufs=2, space="PSUM") as pscore,
        ):
            xqh = big.tile([P, NT, S], f16)
            xql = big.tile([P, NT, S], f16)
            xmh = big.tile([P, NT, MPAD], f16)
            xml = big.tile([P, NT, MPAD], f16)
            merged = big.tile([P, PAIRS, S], f32)    # gathered V^T rows
            stage = big.tile([P, 2 * MYH * 8, 8], u16)  # max_index staging

            with (
                tc.tile_pool(name="w", bufs=2) as wpool,
                tc.tile_pool(name="wo", bufs=1) as wop,
                tc.tile_pool(name="ob", bufs=2) as obp,
            ):
                wts = {}

                def load_pair_weights(pr):
                    csl = slice(pr * P, (pr + 1) * P)
                    t = {}
                    for nm, dr in (("wqh", wqh_d), ("wql", wql_d),
                                   ("wkh", wkh_d), ("wkl", wkl_d),
                                   ("wv", wv_d)):
                        t[nm] = wpool.tile([P, NT, P], f16, tag=nm,
                                           name=f"{nm}{pr}")
                        nc.sync.dma_start(
                            t[nm][:], dr[:, csl].rearrange("(t p) c -> p t c", p=P))
                    wts[pr] = t

                # pair-0 weights split per-dt and interleaved with the x
                # slices in consumption order (K path first).
                csl0 = slice(0, P)
                t0 = {nm: wpool.tile([P, NT, P], f16, tag=nm, name=f"{nm}0")
                      for nm in ("wqh", "wql", "wkh", "wkl", "wv")}
                wts[0] = t0
                for dt in range(NT):
                    dsl = slice(dt * P, (dt + 1) * P)
                    nc.sync.dma_start(xmh[:, dt, :], xmh_d[dsl, :])
                    nc.sync.dma_start(xml[:, dt, :], xml_d[dsl, :])
                    nc.sync.dma_start(t0["wkh"][:, dt, :], wkh_d[dsl, csl0])
                    nc.sync.dma_start(t0["wkl"][:, dt, :], wkl_d[dsl, csl0])
                nc.sync.dma_start(
                    t0["wqh"][:], wqh_d[:, csl0].rearrange("(t p) c -> p t c", p=P))
                nc.sync.dma_start(
                    t0["wql"][:], wql_d[:, csl0].rearrange("(t p) c -> p t c", p=P))
                # xq column-chunked to match Q-projection consumption order
                for ch in range(S // QCHUNK):
                    qsl = slice(ch * QCHUNK, (ch + 1) * QCHUNK)
                    for dt in range(NT):
                        dsl = slice(dt * P, (dt + 1) * P)
                        nc.sync.dma_start(xqh[:, dt, qsl], xqh_d[dsl, qsl])
                        nc.sync.dma_start(xql[:, dt, qsl], xql_d[dsl, qsl])
                nc.sync.dma_start(
                    t0["wv"][:], wv_d[:, csl0].rearrange("(t p) c -> p t c", p=P))

                wot = wop.tile([P, PAIRS, D], f16)
                for pr in range(PAIRS):
                    nc.sync.dma_start(wot[:, pr, :], wo_d[pr * P:(pr + 1) * P, :])

                with (
                    tc.tile_pool(name="qst", bufs=2) as qstp,
                    tc.tile_pool(name="ka", bufs=2) as kap,
                    tc.tile_pool(name="kb", bufs=2) as kbp,
                    tc.tile_pool(name="lo", bufs=2) as lop,
                    tc.tile_pool(name="hi32", bufs=2) as h32p,
                    tc.tile_pool(name="vt", bufs=2) as vtp,
                    tc.tile_pool(name="m8", bufs=8) as m8p,
                    tc.tile_pool(name="idx", bufs=2) as idxp,
                ):
                    qst_t, ka_t, kb_t, vts_t = {}, {}, {}, {}

                    def proj_chunks(pr):
                        """Per-chunk emitters for pair pr's K/Q/V projections
                        (8 chunks).  Q/K evacuate as swapped fp16 hi/lo
                        stacks; hi = fp16(psum) (Act cast), lo = fp16(psum -
                        f32(hi)) (DVE subtract at pair granularity)."""
                        w = wts[pr]
                        qst = [qstp.tile([P, S], f16, tag=f"qst{sh}",
                                         name=f"qst{sh}_{pr}") for sh in range(2)]
                        ka = [kap.tile([P, MPAD], f16, tag=f"ka{sh}",
                                       name=f"ka{sh}_{pr}") for sh in range(2)]
                        kb = [kbp.tile([P, MPAD], f16, tag=f"kb{sh}",
                                       name=f"kb{sh}_{pr}") for sh in range(2)]
                        vts = vtp.tile([P, MPAD], f32, tag="vts", name=f"vts{pr}")
                        qst_t[pr], ka_t[pr], kb_t[pr] = qst, ka, kb
                        vts_t[pr] = vts
                        HALF = [slice(0, DEPTH), slice(DEPTH, P)]

                        def mm3(psum, wh, wl, xh, xl, cw, sl, dts):
                            for dt in dts:
                                nc.tensor.matmul(psum[:, 0:cw], wh[:, dt, :],
                                                 xh[:, dt, sl],
                                                 start=(dt == 0), stop=False)
                                nc.tensor.matmul(psum[:, 0:cw], wh[:, dt, :],
                                                 xl[:, dt, sl],
                                                 start=False, stop=False)
                                nc.tensor.matmul(psum[:, 0:cw], wl[:, dt, :],
                                                 xh[:, dt, sl],
                                                 start=False, stop=(dt == NT - 1))

                        def k_split(pk, cw, ksl):
                            h32 = h32p.tile([P, 512], f32, tag="h32")
                            lo = lop.tile([P, 512], f16, tag="lo")
                            for sh in range(2):
                                # hi into stack A top / stack B bottom
                                nc.scalar.copy(ka[sh][HALF[0], ksl],
                                               pk[HALF[sh], 0:cw])
                                nc.scalar.copy(kb[sh][HALF[1], ksl],
                                               pk[HALF[sh], 0:cw])
                                nc.scalar.copy(h32[HALF[sh], 0:cw],
                                               ka[sh][HALF[0], ksl])
                            nc.vector.tensor_sub(lo[:, 0:cw], pk[:, 0:cw],
                                                 h32[:, 0:cw])
                            for sh in range(2):
                                nc.gpsimd.tensor_copy(ka[sh][HALF[1], ksl],
                                                      lo[HALF[sh], 0:cw])
                                nc.gpsimd.tensor_copy(kb[sh][HALF[0], ksl],
                                                      lo[HALF[sh], 0:cw])

                        def k_chunk(co, cw):
                            # two pop-able halves so PE filler stays ~2.5us
                            ksl = slice(co, co + cw)
                            pk = pproj.tile([P, 512], f32, tag="pk")
                            mm3(pk, w["wkh"], w["wkl"], xmh, xml, cw, ksl,
                                range(NT // 2))

                            def second():
                                mm3(pk, w["wkh"], w["wkl"], xmh, xml, cw, ksl,
                                    range(NT // 2, NT))
                                k_split(pk, cw, ksl)
                            return second

                        def q_split(pq, cw, qsl):
                            h32 = h32p.tile([P, 512], f32, tag="h32")
                            lo = lop.tile([P, 512], f16, tag="lo")
                            for sh in range(2):
                                nc.scalar.copy(qst[sh][HALF[0], qsl],
                                               pq[HALF[sh], 0:cw])
                                nc.scalar.copy(h32[HALF[sh], 0:cw],
                                               qst[sh][HALF[0], qsl])
                            nc.vector.tensor_sub(lo[:, 0:cw], pq[:, 0:cw],
                                                 h32[:, 0:cw])
                            for sh in range(2):
                                nc.gpsimd.tensor_copy(qst[sh][HALF[1], qsl],
                                                      lo[HALF[sh], 0:cw])

                        def q_chunk(ch):
                            qsl = slice(ch * QCHUNK, (ch + 1) * QCHUNK)
                            cw = QCHUNK
                            pq = pproj.tile([P, 512], f32, tag="pk")
                            mm3(pq, w["wqh"], w["wql"], xqh, xql, cw, qsl,
                                range(NT // 2))

                            def second():
                                mm3(pq, w["wqh"], w["wql"], xqh, xql, cw, qsl,
                                    range(NT // 2, NT))
                                q_split(pq, cw, qsl)
                            return second

                        def v_chunk(co, cw):
                            ksl = slice(co, co + cw)
                            pv = pproj.tile([P, 512], f32, tag="pk")
                            for dt in range(NT):
                                nc.tensor.matmul(pv[:, 0:cw], w["wv"][:, dt, :],
                                                 xmh[:, dt, ksl],
                                                 start=(dt == 0), stop=(dt == NT - 1))
                            nc.scalar.copy(vts[:, ksl], pv[:, 0:cw])
                            return None

                        def halves(fn, *args):
                            # emitter pair: first half now-able, second closure
                            done = {}

                            def first():
                                done["s"] = fn(*args)

                            def second():
                                done.pop("s")()
                            return [first, second]

                        emitters = []
                        for co, cw in KCHUNKS:
                            emitters += halves(k_chunk, co, cw)
                        for ch in range(S // QCHUNK):
                            emitters += halves(q_chunk, ch)
                        for co, cw in KCHUNKS:
                            emitters.append(lambda co=co, cw=cw: v_chunk(co, cw))
                        return emitters

                    idxts = {}

                    def score_head(pr, sh, pending):
                        """16 score+argmax tiles for head (pr, sh), draining
                        `pending` proj-chunk emitters between tiles; then
                        bounce this head's indices through DRAM."""
                        h = pr * 2 + sh
                        qst = qst_t[pr][sh]
                        ka, kb = ka_t[pr][sh], kb_t[pr][sh]
                        last = (pr == PAIRS - 1 and sh == 1)
                        for t in range(QTILES):
                            col = h * QTILES + t
                            tsl = slice(t * P, (t + 1) * P)
                            sc = pscore.tile([P, MPAD], f32, tag="sc", bufs=3)
                            m8 = m8p.tile([P, 8], f32, tag="m8")
                            for ci, (co, cw) in enumerate(KCHUNKS):
                                ksl = slice(co, co + cw)
                                nc.tensor.matmul(sc[:, ksl], qst[:, tsl],
                                                 ka[:, ksl],
                                                 start=True, stop=False)
                                nc.tensor.matmul(sc[:, ksl], qst[:, tsl],
                                                 kb[:, ksl],
                                                 start=False, stop=True)
                            nc.vector.max(m8[:], sc[:, 0:MPAD])
                            nc.vector.max_index(stage[:, col, :],
                                                m8[:], sc[:, 0:MPAD])
                            if last and t == QTILES // 2 - 1:
                                # early half-bounce: queries 0..1023 of the
                                # final head overlap the second half's tiles
                                nc.sync.dma_start(
                                    kidx[h, 0:S // 2].rearrange("(t pp) -> pp t", pp=P),
                                    stage[:, h * QTILES:h * QTILES + QTILES // 2, 0])
                                for r in range(4):
                                    dst = idxts[pr][sh * 64 + r * 16: sh * 64 + (r + 1) * 16, 0:64]
                                    nc.sync.dma_start(
                                        dst, kidx[h, 0:S // 2].rearrange("(c pp) -> pp c", pp=16).bitcast(i16))
                                # first-half gather + first-half out3 overlap
                                # the remaining 8 score tiles
                                nc.gpsimd.ap_gather(
                                    merged[:, pr, 0:S // 2], vts_t[pr][:],
                                    idxts[pr][:, 0:64],
                                    channels=P, num_elems=MPAD, d=1,
                                    num_idxs=S // 2)
                                pending.extend(out_proj_pass(
                                    out3, (3,), range(QTILES // 2)))
                            if pending and (last or t % 2 == 1 or len(pending) > 12):
                                pending.pop(0)()
                        # store this head's indices, reload 16-wrapped
                        if last:
                            nc.sync.dma_start(
                                kidx[h, S // 2:].rearrange("(t pp) -> pp t", pp=P),
                                stage[:, h * QTILES + QTILES // 2:(h + 1) * QTILES, 0])
                            for r in range(4):
                                dst = idxts[pr][sh * 64 + r * 16: sh * 64 + (r + 1) * 16, 64:128]
                                nc.sync.dma_start(
                                    dst, kidx[h, S // 2:].rearrange("(c pp) -> pp c", pp=16).bitcast(i16))
                        else:
                            nc.sync.dma_start(
                                kidx[h, :].rearrange("(t pp) -> pp t", pp=P),
                                stage[:, h * QTILES:(h + 1) * QTILES, 0])
                            for r in range(4):
                                dst = idxts[pr][sh * 64 + r * 16: sh * 64 + (r + 1) * 16, :]
                                nc.sync.dma_start(
                                    dst, kidx[h, :].rearrange("(c pp) -> pp c", pp=16).bitcast(i16))

                    with tc.tile_pool(name="mst", bufs=2) as mstp:

                        def out_proj_pass(dst, prs, trange=None):
                            """Partial output projection over pair set `prs`
                            (bounced to DRAM; host sums the partials)."""
                            emitters = []
                            for t in (trange if trange is not None
                                      else range(QTILES)):
                                def emit(t=t):
                                    po = pscore.tile([P, D], f32, tag="sc", bufs=3)
                                    mt = mstp.tile([P, len(prs), P], f16, tag="mst")
                                    for i, pr in enumerate(prs):
                                        nc.scalar.copy(mt[:, i, :],
                                                       merged[:, pr, t * P:(t + 1) * P])
                                    for i, pr in enumerate(prs):
                                        st, sp = (i == 0), (i == len(prs) - 1)
                                        for hf in range(2):
                                            osl = slice(hf * 512, (hf + 1) * 512)
                                            nc.tensor.matmul(po[:, osl], mt[:, i, :],
                                                             wot[:, pr, osl],
                                                             start=st, stop=sp)
                                    ob = obp.tile([P, D], f16, tag="ob")
                                    nc.scalar.copy(ob[:], po[:])
                                    nc.sync.dma_start(dst[t * P:(t + 1) * P, :], ob[:])
                                emitters.append(emit)
                            return emitters

                        # pair-0 projections: K and first-Q eagerly (tile 0
                        # of head 0 needs all keys but only queries 0..511);
                        # the rest drain inside the first head's score loop.
                        # eager: both K chunks + first Q chunk (tile 0 of
                        # head 0 needs all keys but only queries 0..511)
                        pending = proj_chunks(0)
                        for _ in range(6):
                            pending.pop(0)()
                        for pr in range(PAIRS):
                            if pr + 1 < PAIRS:
                                load_pair_weights(pr + 1)
                                pending += proj_chunks(pr + 1)
                            if pr == 2:
                                # pairs 0+1 gathered: emit their partial
                                # out-projection into the mid-kernel gaps
                                pending += out_proj_pass(out01, (0, 1))
                            if pr == 3:
                                pending += out_proj_pass(out2, (2,))
                            idxts[pr] = idxp.tile([P, S // 16], i16, tag="idxt", name=f"idxt{pr}")
                            score_head(pr, 0, pending)
                            score_head(pr, 1, pending)
                            if pr < PAIRS - 1:
                                nc.gpsimd.ap_gather(
                                    merged[:, pr, :], vts_t[pr][:], idxts[pr][:],
                                    channels=P, num_elems=MPAD, d=1, num_idxs=S)
                        # tail: second-half gather + remaining out3 qtiles
                        nc.gpsimd.ap_gather(
                            merged[:, 3, S // 2:], vts_t[3][:],
                            idxts[3][:, 64:128],
                            channels=P, num_elems=MPAD, d=1, num_idxs=S // 2)
                        for e in pending:
                            e()
                        for e in out_proj_pass(out3, (3,), range(QTILES // 2, QTILES)):
                            e()

    nc.compile()
    return nc


def _split16(a):
    """Exact-ish fp16 Veltkamp split: a ~= hi + lo, both fp16 (RNE)."""
    import ml_dtypes  # noqa: F401  (np.float16 is native; kept for parity)
    hi = a.astype(np.float16)
    lo = (a.astype(np.float32) - hi.astype(np.float32)).astype(np.float16)
    return hi, lo


def prep_core_inputs(c, x, mask, W_q, W_k, W_v, W_o, b_o, gamma, beta,
                     moving_mean, moving_var):
    """Host-side per-core input prep: head-split sharding, fp16 hi/lo
    plane splits, argmax-safe x1024 scaling of W_q/W_k, BN fold."""
    b, hg = c // 2, c % 2
    csl = slice(hg * MYD, (hg + 1) * MYD)
    xb = np.asarray(x[b], dtype=np.float32)

    midx = np.where(np.asarray(mask[b, 0, 0]) == 0)[0]
    assert len(midx) > 0
    kept = midx[:MPAD]
    xm = np.zeros((MPAD, D), dtype=np.float32)
    xm[:len(kept)] = xb[kept, :]

    s = np.asarray(gamma, np.float64) / np.sqrt(np.asarray(moving_var, np.float64) + BN_EPS)
    wo_f = (np.asarray(W_o, np.float64)[csl, :] * s[None, :]).astype(np.float32)

    wq_s = (np.asarray(W_q, np.float64)[:, csl] * (-1024.0)).astype(np.float32)
    wk_s = (np.asarray(W_k, np.float64)[:, csl] * 1024.0).astype(np.float32)

    xqT = np.ascontiguousarray(xb.T)
    xmT = np.ascontiguousarray(xm.T)
    xqh, xql = _split16(xqT)
    xmh, xml = _split16(xmT)
    wqh, wql = _split16(wq_s)
    wkh, wkl = _split16(wk_s)
    return {
        "xqh": xqh, "xql": xql, "xmh": xmh, "xml": xml,
        "wqh": np.ascontiguousarray(wqh), "wql": np.ascontiguousarray(wql),
        "wkh": np.ascontiguousarray(wkh), "wkl": np.ascontiguousarray(wkl),
        "wv": np.ascontiguousarray(np.asarray(W_v, np.float32)[:, csl]).astype(np.float16),
        "wo": np.ascontiguousarray(wo_f).astype(np.float16),
    }


def host_bias(b_o, gamma, beta, moving_mean, moving_var):
    s = np.asarray(gamma, np.float64) / np.sqrt(np.asarray(moving_var, np.float64) + BN_EPS)
    return ((np.asarray(b_o, np.float64) - np.asarray(moving_mean, np.float64)) * s
            + np.asarray(beta, np.float64)).astype(np.float32)


_NC_CACHE = None


def _get_nc():
    global _NC_CACHE
    if _NC_CACHE is None:
        _NC_CACHE = build()
    return _NC_CACHE


def combine_outputs(results, inputs):
    """Partial sums -> full output, with host-side patching of rows whose
    true argmax is among the (<=12 per batch) masked keys dropped from the
    on-device 1024-key subset."""
    bias = host_bias(inputs["b_o"], inputs["gamma"], inputs["beta"],
                     inputs["moving_mean"], inputs["moving_var"])
    s_bn = (np.asarray(inputs["gamma"], np.float64)
            / np.sqrt(np.asarray(inputs["moving_var"], np.float64) + BN_EPS))
    Wq = np.asarray(inputs["W_q"], np.float64)
    Wk = np.asarray(inputs["W_k"], np.float64)
    Wv = np.asarray(inputs["W_v"], np.float64)
    Wo = np.asarray(inputs["W_o"], np.float64) * s_bn[None, :]
    out = np.zeros((B, S, D), dtype=np.float32)
    for b in range(B):
        acc = bias.astype(np.float64)[None, :] + sum(
            results[2 * b + hg][nm].astype(np.float64)
            for hg in range(2) for nm in ("out01", "out2", "out3"))
        midx = np.where(np.asarray(inputs["mask"][b, 0, 0]) == 0)[0]
        if len(midx) > MPAD:
            kept = midx[:MPAD]
            dropped = midx[MPAD:]
            xb = np.asarray(inputs["x"][b], np.float64)
            Qb = xb @ Wq                      # [S, D]
            Kk = xb[kept] @ Wk                # [MPAD, D]
            Kd = xb[dropped] @ Wk             # [nd, D]
            for hg in range(2):
                ki = results[2 * b + hg]["kidx"]  # [MYH, S] device argmax
                for hh in range(MYH):
                    h = hg * MYH + hh
                    sl = slice(h * DEPTH, (h + 1) * DEPTH)
                    qh = Qb[:, sl]
                    # device winner's score vs dropped keys' scores (f64)
                    s_dev = -np.einsum("qd,qd->q", qh,
                                       Kk[ki[hh].astype(np.int64), sl]) / 32.0
                    s_drop = -(qh @ Kd[:, sl].T) / 32.0  # [S, nd]
                    jbest = s_drop.argmax(axis=1)
                    better = s_drop[np.arange(S), jbest] > s_dev
                    for q in np.nonzero(better)[0]:
                        k_new = dropped[jbest[q]]
                        k_old = kept[ki[hh, q]]
                        dv = (xb[k_new] - xb[k_old])[None, :] @ Wv[:, sl]
                        acc[q] += (dv @ Wo[sl, :])[0]
        out[b] = acc.astype(np.float32)
    return out


def kernel(**inputs) -> np.ndarray:
    nc = _get_nc()
    in_maps = [prep_core_inputs(c, **inputs) for c in range(NCORES)]
    res = run_bass_kernel_spmd(nc, in_maps, list(range(NCORES)))
    return combine_outputs(res.results, inputs)


# revision 27
# speedup vs baseline: 1.0215x; 1.0215x over previous
"""MultiHeadAttention + BatchNorm (inference) Trainium2 Bass kernel, v3.

Same one-hot-softmax argmax-gather algorithm as v2 (the reference's
multiplicative mask makes softmax an exact one-hot on the most-negative
masked-key score), rebuilt around two PE tricks:

  * All projections run as 3-term fp16 Dekker matmuls (xh*wh + xh*wl +
    xl*wh, hi/lo splits computed on the HOST): 24 fp16 matmuls per
    128-col pair replace 8 fp32 matmuls -- 3 PE cycles/row vs 4.
    W_q/W_k are pre-scaled by 1024 (argmax is scale-invariant) so their
    fp16 lo-planes stay normal-range.
  * Scores use a swapped-stack EXACT product: per head, Q is stored as
    [qh;ql] (fp16 hi over lo, 128 partitions) and K twice as [kh;kl]
    and [kl;kh].  Two matmuls against the two K stacks accumulate
    qh*kh + ql*kl + qh*kl + ql*kh = the full (qh+ql)(kh+kl) product --
    2 fp16 matmuls per 512-key chunk instead of v2's 3, at BETTER
    precision (the ql*kl term is kept, not dropped).
  * V / output-projection path runs in fp16 instead of bf16 (same PE
    cost, 8x finer mantissa): final rel err ~8e-4 vs v2's 3.2e-3.

Per-engine budget (TimelineSim cost model): PE 273us, DVE 286us
(max/max_index argmax scan on PSUM is irreducible), Act ~100us.
"""
import numpy as np

import concourse.bass as bass
import concourse.tile as tile
from concourse import bacc, mybir
from concourse.bass_utils import run_bass_kernel_spmd

f32 = mybir.dt.float32
f16 = mybir.dt.float16
u16 = mybir.dt.uint16
i16 = mybir.dt.int16

B, S, D, H = 4, 2048, 1024, 16
DEPTH = D // H          # 64
P = 128
NCORES = 8
MYH = H // 2            # heads per core (8)
MYD = MYH * DEPTH       # 512 output dims of my heads
NT = D // P             # contraction tiles (8)
PAIRS = MYH // 2        # head pairs per core (4)
QTILES = S // P         # 16
# Only the first 1024 masked keys are scored on-device (see v2 docstring);
# overflow keys (<=12, batch 3 only) are checked on the host in f64.
MPAD = 1024
KCHUNKS = [(0, 512), (512, 512)]
QCHUNK = 512
BN_EPS = 1e-3


def build():
    nc = bacc.Bacc(None, target_bir_lowering=False, debug=False,
                   dynamic_dma_scratch_size=2048)

    xqh_d = nc.dram_tensor("xqh", [D, S], f16, kind="ExternalInput")
    xql_d = nc.dram_tensor("xql", [D, S], f16, kind="ExternalInput")
    xmh_d = nc.dram_tensor("xmh", [D, MPAD], f16, kind="ExternalInput")
    xml_d = nc.dram_tensor("xml", [D, MPAD], f16, kind="ExternalInput")
    wqh_d = nc.dram_tensor("wqh", [D, MYD], f16, kind="ExternalInput")  # -1024*W_q hi
    wql_d = nc.dram_tensor("wql", [D, MYD], f16, kind="ExternalInput")
    wkh_d = nc.dram_tensor("wkh", [D, MYD], f16, kind="ExternalInput")  # 1024*W_k hi
    wkl_d = nc.dram_tensor("wkl", [D, MYD], f16, kind="ExternalInput")
    wv_d = nc.dram_tensor("wv", [D, MYD], f16, kind="ExternalInput")
    wo_d = nc.dram_tensor("wo", [MYD, D], f16, kind="ExternalInput")  # BN scale folded
    out01 = nc.dram_tensor("out01", [S, D], f16, kind="ExternalOutput")
    out2 = nc.dram_tensor("out2", [S, D], f16, kind="ExternalOutput")
    out3 = nc.dram_tensor("out3", [S, D], f16, kind="ExternalOutput")
    kidx = nc.dram_tensor("kidx", [MYH, S], u16, kind="ExternalOutput")

    with tile.TileContext(nc) as tc:
        with (
            tc.tile_pool(name="big", bufs=1) as big,
            tc.tile_pool(name="pproj", bufs=2, space="PSUM") as pproj,
            tc.tile_pool(name="pscore", bufs=2, space="PSUM") as pscore,
        ):
            xqh = big.tile([P, NT, S], f16)
            xql = big.tile([P, NT, S], f16)
            xmh = big.tile([P, NT, MPAD], f16)
            xml = big.tile([P, NT, MPAD], f16)
            merged = big.tile([P, PAIRS, S], f32)    # gathered V^T rows
            stage = big.tile([P, 2 * MYH * 8, 8], u16)  # max_index staging

            with (
                tc.tile_pool(name="w", bufs=2) as wpool,
                tc.tile_pool(name="wo", bufs=1) as wop,
                tc.tile_pool(name="ob", bufs=3) as obp,
            ):
                wts = {}

                def load_pair_weights(pr):
                    csl = slice(pr * P, (pr + 1) * P)
                    t = {}
                    for nm, dr in (("wqh", wqh_d), ("wql", wql_d),
                                   ("wkh", wkh_d), ("wkl", wkl_d),
                                   ("wv", wv_d)):
                        t[nm] = wpool.tile([P, NT, P], f16, tag=nm,
                                           name=f"{nm}{pr}")
                        nc.sync.dma_start(
                            t[nm][:], dr[:, csl].rearrange("(t p) c -> p t c", p=P))
                    wts[pr] = t

                # pair-0 weights split per-dt and interleaved with the x
                # slices in consumption order (K path first).
                csl0 = slice(0, P)
                t0 = {nm: wpool.tile([P, NT, P], f16, tag=nm, name=f"{nm}0")
                      for nm in ("wqh", "wql", "wkh", "wkl", "wv")}
                wts[0] = t0
                for dt in range(NT):
                    dsl = slice(dt * P, (dt + 1) * P)
                    nc.sync.dma_start(xmh[:, dt, :], xmh_d[dsl, :])
                    nc.sync.dma_start(xml[:, dt, :], xml_d[dsl, :])
                    nc.sync.dma_start(t0["wkh"][:, dt, :], wkh_d[dsl, csl0])
                    nc.sync.dma_start(t0["wkl"][:, dt, :], wkl_d[dsl, csl0])
                nc.sync.dma_start(
                    t0["wqh"][:], wqh_d[:, csl0].rearrange("(t p) c -> p t c", p=P))
                nc.sync.dma_start(
                    t0["wql"][:], wql_d[:, csl0].rearrange("(t p) c -> p t c", p=P))
                # xq column-chunked to match Q-projection consumption order
                for ch in range(S // QCHUNK):
                    qsl = slice(ch * QCHUNK, (ch + 1) * QCHUNK)
                    for dt in range(NT):
                        dsl = slice(dt * P, (dt + 1) * P)
                        nc.sync.dma_start(xqh[:, dt, qsl], xqh_d[dsl, qsl])
                        nc.sync.dma_start(xql[:, dt, qsl], xql_d[dsl, qsl])
                nc.sync.dma_start(
                    t0["wv"][:], wv_d[:, csl0].rearrange("(t p) c -> p t c", p=P))

                wot = wop.tile([P, PAIRS, D], f16)
                for pr in range(PAIRS):
                    nc.sync.dma_start(wot[:, pr, :], wo_d[pr * P:(pr + 1) * P, :])

                with (
                    tc.tile_pool(name="qst", bufs=2) as qstp,
                    tc.tile_pool(name="ka", bufs=2) as kap,
                    tc.tile_pool(name="kb", bufs=2) as kbp,
                    tc.tile_pool(name="lo", bufs=2) as lop,
                    tc.tile_pool(name="hi32", bufs=2) as h32p,
                    tc.tile_pool(name="vt", bufs=2) as vtp,
                    tc.tile_pool(name="m8", bufs=8) as m8p,
                    tc.tile_pool(name="idx", bufs=2) as idxp,
                ):
                    qst_t, ka_t, kb_t, vts_t = {}, {}, {}, {}

                    def proj_chunks(pr):
                        """Per-chunk emitters for pair pr's K/Q/V projections
                        (8 chunks).  Q/K evacuate as swapped fp16 hi/lo
                        stacks; hi = fp16(psum) (Act cast), lo = fp16(psum -
                        f32(hi)) (DVE subtract at pair granularity)."""
                        w = wts[pr]
                        qst = [qstp.tile([P, S], f16, tag=f"qst{sh}",
                                         name=f"qst{sh}_{pr}") for sh in range(2)]
                        ka = [kap.tile([P, MPAD], f16, tag=f"ka{sh}",
                                       name=f"ka{sh}_{pr}") for sh in range(2)]
                        kb = [kbp.tile([P, MPAD], f16, tag=f"kb{sh}",
                                       name=f"kb{sh}_{pr}") for sh in range(2)]
                        vts = vtp.tile([P, MPAD], f32, tag="vts", name=f"vts{pr}")
                        qst_t[pr], ka_t[pr], kb_t[pr] = qst, ka, kb
                        vts_t[pr] = vts
                        HALF = [slice(0, DEPTH), slice(DEPTH, P)]

                        def mm3(psum, wh, wl, xh, xl, cw, sl, dts):
                            for dt in dts:
                                nc.tensor.matmul(psum[:, 0:cw], wh[:, dt, :],
                                                 xh[:, dt, sl],
                                                 start=(dt == 0), stop=False)
                                nc.tensor.matmul(psum[:, 0:cw], wh[:, dt, :],
                                                 xl[:, dt, sl],
                                                 start=False, stop=False)
                                nc.tensor.matmul(psum[:, 0:cw], wl[:, dt, :],
                                                 xh[:, dt, sl],
                                                 start=False, stop=(dt == NT - 1))

                        def k_split(pk, cw, ksl):
                            h32 = h32p.tile([P, 512], f32, tag="h32")
                            lo = lop.tile([P, 512], f16, tag="lo")
                            for sh in range(2):
                                # hi into stack A top / stack B bottom
                                nc.scalar.copy(ka[sh][HALF[0], ksl],
                                               pk[HALF[sh], 0:cw])
                                nc.scalar.copy(kb[sh][HALF[1], ksl],
                                               pk[HALF[sh], 0:cw])
                                nc.scalar.copy(h32[HALF[sh], 0:cw],
                                               ka[sh][HALF[0], ksl])
                            nc.vector.tensor_sub(lo[:, 0:cw], pk[:, 0:cw],
                                                 h32[:, 0:cw])
                            for sh in range(2):
                                nc.gpsimd.tensor_copy(ka[sh][HALF[1], ksl],
                                                      lo[HALF[sh], 0:cw])
                                nc.gpsimd.tensor_copy(kb[sh][HALF[0], ksl],
                                                      lo[HALF[sh], 0:cw])

                        def k_chunk(co, cw):
                            # two pop-able halves so PE filler stays ~2.5us
                            ksl = slice(co, co + cw)
                            pk = pproj.tile([P, 512], f32, tag="pk")
                            mm3(pk, w["wkh"], w["wkl"], xmh, xml, cw, ksl,
                                range(NT // 2))

                            def second():
                                mm3(pk, w["wkh"], w["wkl"], xmh, xml, cw, ksl,
                                    range(NT // 2, NT))
                                k_split(pk, cw, ksl)
                            return second

                        def q_split(pq, cw, qsl):
                            h32 = h32p.tile([P, 512], f32, tag="h32")
                            lo = lop.tile([P, 512], f16, tag="lo")
                            for sh in range(2):
                                nc.scalar.copy(qst[sh][HALF[0], qsl],
                                               pq[HALF[sh], 0:cw])
                                nc.scalar.copy(h32[HALF[sh], 0:cw],
                                               qst[sh][HALF[0], qsl])
                            nc.vector.tensor_sub(lo[:, 0:cw], pq[:, 0:cw],
                                                 h32[:, 0:cw])
                            for sh in range(2):
                                nc.gpsimd.tensor_copy(qst[sh][HALF[1], qsl],
                                                      lo[HALF[sh], 0:cw])

                        def q_chunk(ch):
                            qsl = slice(ch * QCHUNK, (ch + 1) * QCHUNK)
                            cw = QCHUNK
                            pq = pproj.tile([P, 512], f32, tag="pk")
                            mm3(pq, w["wqh"], w["wql"], xqh, xql, cw, qsl,
                                range(NT // 2))

                            def second():
                                mm3(pq, w["wqh"], w["wql"], xqh, xql, cw, qsl,
                                    range(NT // 2, NT))
                                q_split(pq, cw, qsl)
                            return second

                        def v_chunk(co, cw):
                            ksl = slice(co, co + cw)
                            pv = pproj.tile([P, 512], f32, tag="pk")
                            for dt in range(NT):
                                nc.tensor.matmul(pv[:, 0:cw], w["wv"][:, dt, :],
                                                 xmh[:, dt, ksl],
                                                 start=(dt == 0), stop=(dt == NT - 1))
                            nc.scalar.copy(vts[:, ksl], pv[:, 0:cw])
                            return None

                        def halves(fn, *args):
                            # emitter pair: first half now-able, second closure
                            done = {}

                            def first():
                                done["s"] = fn(*args)

                            def second():
                                done.pop("s")()
                            return [first, second]

                        emitters = []
                        for co, cw in KCHUNKS:
                            emitters += halves(k_chunk, co, cw)
                        for ch in range(S // QCHUNK):
                            emitters += halves(q_chunk, ch)
                        for co, cw in KCHUNKS:
                            emitters.append(lambda co=co, cw=cw: v_chunk(co, cw))
                        return emitters

                    idxts = {}

                    def score_head(pr, sh, pending):
                        """16 score+argmax tiles for head (pr, sh), draining
                        `pending` proj-chunk emitters between tiles; then
                        bounce this head's indices through DRAM."""
                        h = pr * 2 + sh
                        qst = qst_t[pr][sh]
                        ka, kb = ka_t[pr][sh], kb_t[pr][sh]
                        last = (pr == PAIRS - 1 and sh == 1)
                        for t in range(QTILES):
                            col = h * QTILES + t
                            tsl = slice(t * P, (t + 1) * P)
                            sc = pscore.tile([P, MPAD], f32, tag="sc", bufs=3)
                            m8 = m8p.tile([P, 8], f32, tag="m8")
                            for ci, (co, cw) in enumerate(KCHUNKS):
                                ksl = slice(co, co + cw)
                                nc.tensor.matmul(sc[:, ksl], qst[:, tsl],
                                                 ka[:, ksl],
                                                 start=True, stop=False)
                                nc.tensor.matmul(sc[:, ksl], qst[:, tsl],
                                                 kb[:, ksl],
                                                 start=False, stop=True)
                            nc.vector.max(m8[:], sc[:, 0:MPAD])
                            nc.vector.max_index(stage[:, col, :],
                                                m8[:], sc[:, 0:MPAD])
                            if last and t == QTILES // 2 - 1:
                                # early half-bounce: queries 0..1023 of the
                                # final head overlap the second half's tiles
                                nc.sync.dma_start(
                                    kidx[h, 0:S // 2].rearrange("(t pp) -> pp t", pp=P),
                                    stage[:, h * QTILES:h * QTILES + QTILES // 2, 0])
                                for r in range(4):
                                    dst = idxts[pr][sh * 64 + r * 16: sh * 64 + (r + 1) * 16, 0:64]
                                    nc.sync.dma_start(
                                        dst, kidx[h, 0:S // 2].rearrange("(c pp) -> pp c", pp=16).bitcast(i16))
                                # first-half gather + first-half out3 overlap
                                # the remaining 8 score tiles
                                nc.gpsimd.ap_gather(
                                    merged[:, pr, 0:S // 2], vts_t[pr][:],
                                    idxts[pr][:, 0:64],
                                    channels=P, num_elems=MPAD, d=1,
                                    num_idxs=S // 2)
                                pending.extend(out_proj_pass(
                                    out3, (3,), range(QTILES // 2)))
                            if pending and (last or t % 2 == 1 or len(pending) > 12):
                                pending.pop(0)()
                        # store this head's indices, reload 16-wrapped
                        if last:
                            nc.sync.dma_start(
                                kidx[h, S // 2:].rearrange("(t pp) -> pp t", pp=P),
                                stage[:, h * QTILES + QTILES // 2:(h + 1) * QTILES, 0])
                            for r in range(4):
                                dst = idxts[pr][sh * 64 + r * 16: sh * 64 + (r + 1) * 16, 64:128]
                                nc.sync.dma_start(
                                    dst, kidx[h, S // 2:].rearrange("(c pp) -> pp c", pp=16).bitcast(i16))
                        else:
                            nc.sync.dma_start(
                                kidx[h, :].rearrange("(t pp) -> pp t", pp=P),
                                stage[:, h * QTILES:(h + 1) * QTILES, 0])
                            for r in range(4):
                                dst = idxts[pr][sh * 64 + r * 16: sh * 64 + (r + 1) * 16, :]
                                nc.sync.dma_start(
                                    dst, kidx[h, :].rearrange("(c pp) -> pp c", pp=16).bitcast(i16))

                    with tc.tile_pool(name="mst", bufs=3) as mstp:

                        def out_proj_pass(dst, prs, trange=None):
                            """Partial output projection over pair set `prs`
                            (bounced to DRAM; host sums the partials)."""
                            emitters = []
                            for t in (trange if trange is not None
                                      else range(QTILES)):
                                def emit(t=t):
                                    po = pscore.tile([P, D], f32, tag="sc", bufs=3)
                                    mt = mstp.tile([P, len(prs), P], f16, tag="mst")
                                    for i, pr in enumerate(prs):
                                        nc.scalar.copy(mt[:, i, :],
                                                       merged[:, pr, t * P:(t + 1) * P])
                                    for i, pr in enumerate(prs):
                                        st, sp = (i == 0), (i == len(prs) - 1)
                                        for hf in range(2):
                                            osl = slice(hf * 512, (hf + 1) * 512)
                                            nc.tensor.matmul(po[:, osl], mt[:, i, :],
                                                             wot[:, pr, osl],
                                                             start=st, stop=sp)
                                    ob = obp.tile([P, D], f16, tag="ob")
                                    nc.scalar.copy(ob[:], po[:])
                                    nc.sync.dma_start(dst[t * P:(t + 1) * P, :], ob[:])
                                emitters.append(emit)
                            return emitters

                        # pair-0 projections: K and first-Q eagerly (tile 0
                        # of head 0 needs all keys but only queries 0..511);
                        # the rest drain inside the first head's score loop.
                        # eager: both K chunks + first Q chunk (tile 0 of
                        # head 0 needs all keys but only queries 0..511)
                        pending = proj_chunks(0)
                        for _ in range(6):
                            pending.pop(0)()
                        for pr in range(PAIRS):
                            if pr + 1 < PAIRS:
                                load_pair_weights(pr + 1)
                                pending += proj_chunks(pr + 1)
                            if pr == 2:
                                # pairs 0+1 gathered: emit their partial
                                # out-projection into the mid-kernel gaps
                                pending += out_proj_pass(out01, (0, 1))
                            if pr == 3:
                                pending += out_proj_pass(out2, (2,))
                            idxts[pr] = idxp.tile([P, S // 16], i16, tag="idxt", name=f"idxt{pr}")
                            score_head(pr, 0, pending)
                            score_head(pr, 1, pending)
                            if pr < PAIRS - 1:
                                nc.gpsimd.ap_gather(
                                    merged[:, pr, :], vts_t[pr][:], idxts[pr][:],
                                    channels=P, num_elems=MPAD, d=1, num_idxs=S)
                        # tail: second-half gather + remaining out3 qtiles
                        nc.gpsimd.ap_gather(
                            merged[:, 3, S // 2:], vts_t[3][:],
                            idxts[3][:, 64:128],
                            channels=P, num_elems=MPAD, d=1, num_idxs=S // 2)
                        for e in pending:
                            e()
                        for e in out_proj_pass(out3, (3,), range(QTILES // 2, QTILES)):
                            e()

    nc.compile()
    return nc


def _split16(a):
    """Exact-ish fp16 Veltkamp split: a ~= hi + lo, both fp16 (RNE)."""
    import ml_dtypes  # noqa: F401  (np.float16 is native; kept for parity)
    hi = a.astype(np.float16)
    lo = (a.astype(np.float32) - hi.astype(np.float32)).astype(np.float16)
    return hi, lo


def prep_core_inputs(c, x, mask, W_q, W_k, W_v, W_o, b_o, gamma, beta,
                     moving_mean, moving_var):
    """Host-side per-core input prep: head-split sharding, fp16 hi/lo
    plane splits, argmax-safe x1024 scaling of W_q/W_k, BN fold."""
    b, hg = c // 2, c % 2
    csl = slice(hg * MYD, (hg + 1) * MYD)
    xb = np.asarray(x[b], dtype=np.float32)

    midx = np.where(np.asarray(mask[b, 0, 0]) == 0)[0]
    assert len(midx) > 0
    kept = midx[:MPAD]
    xm = np.zeros((MPAD, D), dtype=np.float32)
    xm[:len(kept)] = xb[kept, :]

    s = np.asarray(gamma, np.float64) / np.sqrt(np.asarray(moving_var, np.float64) + BN_EPS)
    wo_f = (np.asarray(W_o, np.float64)[csl, :] * s[None, :]).astype(np.float32)

    wq_s = (np.asarray(W_q, np.float64)[:, csl] * (-1024.0)).astype(np.float32)
    wk_s = (np.asarray(W_k, np.float64)[:, csl] * 1024.0).astype(np.float32)

    xqT = np.ascontiguousarray(xb.T)
    xmT = np.ascontiguousarray(xm.T)
    xqh, xql = _split16(xqT)
    xmh, xml = _split16(xmT)
    wqh, wql = _split16(wq_s)
    wkh, wkl = _split16(wk_s)
    return {
        "xqh": xqh, "xql": xql, "xmh": xmh, "xml": xml,
        "wqh": np.ascontiguousarray(wqh), "wql": np.ascontiguousarray(wql),
        "wkh": np.ascontiguousarray(wkh), "wkl": np.ascontiguousarray(wkl),
        "wv": np.ascontiguousarray(np.asarray(W_v, np.float32)[:, csl]).astype(np.float16),
        "wo": np.ascontiguousarray(wo_f).astype(np.float16),
    }


def host_bias(b_o, gamma, beta, moving_mean, moving_var):
    s = np.asarray(gamma, np.float64) / np.sqrt(np.asarray(moving_var, np.float64) + BN_EPS)
    return ((np.asarray(b_o, np.float64) - np.asarray(moving_mean, np.float64)) * s
            + np.asarray(beta, np.float64)).astype(np.float32)


_NC_CACHE = None


def _get_nc():
    global _NC_CACHE
    if _NC_CACHE is None:
        _NC_CACHE = build()
    return _NC_CACHE


def combine_outputs(results, inputs):
    """Partial sums -> full output, with host-side patching of rows whose
    true argmax is among the (<=12 per batch) masked keys dropped from the
    on-device 1024-key subset."""
    bias = host_bias(inputs["b_o"], inputs["gamma"], inputs["beta"],
                     inputs["moving_mean"], inputs["moving_var"])
    s_bn = (np.asarray(inputs["gamma"], np.float64)
            / np.sqrt(np.asarray(inputs["moving_var"], np.float64) + BN_EPS))
    Wq = np.asarray(inputs["W_q"], np.float64)
    Wk = np.asarray(inputs["W_k"], np.float64)
    Wv = np.asarray(inputs["W_v"], np.float64)
    Wo = np.asarray(inputs["W_o"], np.float64) * s_bn[None, :]
    out = np.zeros((B, S, D), dtype=np.float32)
    for b in range(B):
        acc = bias.astype(np.float64)[None, :] + sum(
            results[2 * b + hg][nm].astype(np.float64)
            for hg in range(2) for nm in ("out01", "out2", "out3"))
        midx = np.where(np.asarray(inputs["mask"][b, 0, 0]) == 0)[0]
        if len(midx) > MPAD:
            kept = midx[:MPAD]
            dropped = midx[MPAD:]
            xb = np.asarray(inputs["x"][b], np.float64)
            Qb = xb @ Wq                      # [S, D]
            Kk = xb[kept] @ Wk                # [MPAD, D]
            Kd = xb[dropped] @ Wk             # [nd, D]
            for hg in range(2):
                ki = results[2 * b + hg]["kidx"]  # [MYH, S] device argmax
                for hh in range(MYH):
                    h = hg * MYH + hh
                    sl = slice(h * DEPTH, (h + 1) * DEPTH)
                    qh = Qb[:, sl]
                    # device winner's score vs dropped keys' scores (f64)
                    s_dev = -np.einsum("qd,qd->q", qh,
                                       Kk[ki[hh].astype(np.int64), sl]) / 32.0
                    s_drop = -(qh @ Kd[:, sl].T) / 32.0  # [S, nd]
                    jbest = s_drop.argmax(axis=1)
                    better = s_drop[np.arange(S), jbest] > s_dev
                    for q in np.nonzero(better)[0]:
                        k_new = dropped[jbest[q]]
                        k_old = kept[ki[hh, q]]
                        dv = (xb[k_new] - xb[k_old])[None, :] @ Wv[:, sl]
                        acc[q] += (dv @ Wo[sl, :])[0]
        out[b] = acc.astype(np.float32)
    return out


def kernel(**inputs) -> np.ndarray:
    nc = _get_nc()
    in_maps = [prep_core_inputs(c, **inputs) for c in range(NCORES)]
    res = run_bass_kernel_spmd(nc, in_maps, list(range(NCORES)))
    return combine_outputs(res.results, inputs)
